# revision 38
# baseline (speedup 1.0000x reference)
"""AWQ W4A8 linear (x:[8,32,8192] f32, qweight:[8192,8192] int4-range int32,
w_scales/bias:[8192] f32) -> [8,32,8192] f32 on 8 trn2 NeuronCores.

Column-parallel sharding: qweight / w_scales / bias are split along N
(output channels) across the 8 cores; x — quantized per-token on the host
exactly as the reference does — and the per-token act_scales are
replicated. Each core computes x_q [256,8192] @ qw_shard [8192,1024],
applies the per-token/per-channel dequant + bias epilogue, and writes its
[256,1024] slice; the host concatenates the slices.

Compute strategy: the PE's matmul wall time is set by output-column
cycles — ~(6+FD) cycles per accumulation pass over a [128,512] PSUM tile
regardless of dtype — so the only lever is the contraction width per
pass: 128 rows for bf16, 256 for fp8 DoubleRow (both operands fp8). The
int8 x_q is not fp8-representable, so each pair of adjacent k-chunks
(256 rows) is computed one of two ways:
  - exact pair (2 passes): x_q = hi16 + lo with hi16 = x_q & ~15 (multiples
    of 16 in [-128,112]) and lo in [0,15], both exact in fp8e4m3; one
    DoubleRow pass per plane against the same weight pair.
  - approx pair (1 pass): x8 = e4m3(x_q) rounded on host.
With 16 of the 32 pairs exact (even pairs), the end-to-end rms relative
error is ~1.85e-2 (measured against the reference on the real inputs) and
the PE runs 48 passes/tile instead of 64 — ~41us vs ~55us. Weights are
int4-range, exact in fp8; accumulation is fp32 PSUM, exact for the integer
parts. The output is stored bf16 (adds ~0.1% rms) and upcast on host.

Scheduling: each HWDGE dma_start costs the issuing engine ~0.6-1.5us, so
DMAs are batched: all activation planes (ah/lo interleaved per pair) live
in ONE dram tensor moved in 5 large pieces, and ws/bias/act_scales in one
constants tensor. Weights stream through SBUF slots on the sync queue with
ramped group sizes; activations/constants ride the ACT engine's queue. A
burst of dummy matmuls right after the barrier burns the PE's ~3.4us HAM
cold-clock window while the first DMAs land; the last weight group runs
PSUM-tile-by-tile so the dequant epilogues and output stores overlap the
tail matmuls.
"""

from contextlib import ExitStack

import numpy as np

import concourse.bass as bass
import concourse.mybir as mybir
import concourse.bass_utils as bass_utils
from concourse.dt import dt as cdt

N_CORES = 8
P = 128
B, S, K, N = 8, 32, 8192, 8192
TOK = B * S                      # 256 tokens
NL = N // N_CORES                # 1024 output channels per core
KC = K // P                      # 64 contraction chunks of 128
NPAIR = KC // 2                  # 32 DoubleRow chunk pairs
EPS = 1e-8

# first 16 pairs exact (hi/lo, 2 passes), rest e4m3-approx (1 pass).
# Exact pairs run FIRST: they consume weights at ~140GB/s vs ~280GB/s for
# approx pairs, so the weight stream builds buffer during the startup ramp
# and the cheap approx passes run at the end against fully-resident data.
EXACT_PAIRS = tuple(range(NPAIR // 2))
N_EXACT = len(EXACT_PAIRS)
NPLANE = NPAIR + N_EXACT         # 48 activation planes

# plane index map: ah plane per pair, lo plane right after for exact pairs
AH_IDX = {}
LO_IDX = {}
_i = 0
for _pr in range(NPAIR):
    AH_IDX[_pr] = _i
    _i += 1
    if _pr in EXACT_PAIRS:
        LO_IDX[_pr] = _i
        _i += 1
assert _i == NPLANE

W_GROUPS = [2, 2, 2, 2] + [8] * 7   # weight k-chunks per DMA group
X_GROUPS = [8, 8, 16, 16, 16]       # activation k-chunks per DMA piece
NSLOT = 8                           # weight SBUF slots
N_WARM = 12                         # dummy matmuls to pre-warm the PE clock

assert sum(W_GROUPS) == KC and sum(X_GROUPS) == KC
assert all(g % 2 == 0 for g in W_GROUPS) and all(g % 2 == 0 for g in X_GROUPS)
assert 0 in EXACT_PAIRS and (NPAIR - 1) not in EXACT_PAIRS

_cached = None


def _piece_of(c):
    acc = 0
    for i, gc in enumerate(X_GROUPS):
        if c < acc + gc:
            return i
        acc += gc
    raise ValueError(c)


def _build_nc():
    nc = bass.Bass(
        "TRN2",
        target_bir_lowering=False,
        debug=False,
        enable_asserts=False,
        num_devices=N_CORES,
    )
    dt = mybir.dt
    DR = mybir.MatmulPerfMode.DoubleRow

    xa_d = nc.dram_tensor("xa", [P, NPLANE, 2, TOK], dt.float8e4, kind="ExternalInput")
    qw_d = nc.dram_tensor("qw", [P, KC, NL], dt.float8e4, kind="ExternalInput")
    cst_d = nc.dram_tensor("cst", [P, 2 * NL + 2], dt.float32, kind="ExternalInput")
    out_d = nc.dram_tensor("out", [2, P, NL], dt.bfloat16, kind="ExternalOutput")

    ctx = ExitStack()
    xa_s = ctx.enter_context(nc.sbuf_tensor("xa_s", [P, NPLANE, 2, TOK], dt.float8e4))
    w_s = ctx.enter_context(nc.sbuf_tensor("w_s", [P, NSLOT, 8, NL], dt.float8e4))
    cst_s = ctx.enter_context(nc.sbuf_tensor("cst_s", [P, 2 * NL + 2], dt.float32))
    t_s = ctx.enter_context(nc.sbuf_tensor("t_s", [P, 4, 512], dt.float32))
    o_s = ctx.enter_context(nc.sbuf_tensor("o_s", [P, 4, 512], dt.bfloat16))

    ps = [
        ctx.enter_context(nc.psum_tensor(f"ps{i}", [P, 512], dt.float32))
        for i in range(4)  # (m,n): 00,01,10,11
    ]
    psd = ctx.enter_context(nc.psum_tensor("psd", [P, 512], dt.float32))

    sems = {}

    def sem(name):
        sems[name] = ctx.enter_context(nc.semaphore(name))
        return sems[name]

    s_wg = [sem(f"s_wg{g}") for g in range(len(W_GROUPS))]
    s_xq = [sem(f"s_xq{i}") for i in range(len(X_GROUPS))]
    s_cst = sem("s_cst")
    s_pe = sem("s_pe")
    s_ps = [sem(f"s_ps{i}") for i in range(4)]
    s_ep = [sem(f"s_ep{i}") for i in range(4)]
    s_out = sem("s_out")
    s_dve = sem("s_dve")

    w_starts = np.cumsum([0] + W_GROUPS).tolist()
    x_starts = np.cumsum([0] + X_GROUPS).tolist()
    TILES = [(0, 0), (0, 1), (1, 0), (1, 1)]

    def x_plane_range(i):
        # chunk range [xo, xo+xc) -> plane index range
        xo, xc = x_starts[i], X_GROUPS[i]
        p0, p1 = xo // 2, (xo + xc) // 2
        i0 = AH_IDX[p0]
        i1 = AH_IDX[p1] if p1 < NPAIR else NPLANE
        return i0, i1

    # Issue the critical first weight and activation DMAs before the
    # semaphore-clear barrier so the ~3us DGE spin-up overlaps startup
    # (weights on the sync ring, activations on the ACT ring), and a burst
    # of dummy matmuls on garbage SBUF so the PE's ~3.4us HAM activity
    # window elapses before real work arrives and it runs at 2.4GHz.
    nc.sync.dma_start(
        w_s[:, 0, : W_GROUPS[0], :], qw_d.ap()[:, 0 : W_GROUPS[0], :]
    ).then_inc(s_wg[0], 16)
    i0, i1 = x_plane_range(0)
    nc.scalar.dma_start(
        xa_s[:, i0:i1, :, :], xa_d.ap()[:, i0:i1, :, :]
    ).then_inc(s_xq[0], 16)
    for _ in range(N_WARM):
        nc.tensor.matmul(
            psd.ap(),
            xa_s[:, 0, :, 0:P],
            w_s[:, 0, 0:2, 0:512],
            start=True,
            stop=True,
            perf_mode=mybir.MatmulPerfMode.DoubleRow,
        )

    # Zero our semaphores up front (a previous execution of this NEFF leaves
    # them at their final values), then barrier so no engine runs ahead.
    nums = sorted(s.num for s in sems.values())
    lo = 0
    while lo < len(nums):
        hi = lo
        while hi + 1 < len(nums) and nums[hi + 1] == nums[hi] + 1:
            hi += 1
        nc.gpsimd.sem_clear(range(nums[lo], nums[hi] + 1))
        lo = hi + 1
    nc.all_engine_barrier()

    with nc.Block() as block:

        def w_dma(eng, g):
            gc = W_GROUPS[g]
            if g >= NSLOT:
                eng.wait_ge(s_pe, g - NSLOT + 1)
            c0 = w_starts[g]
            eng.dma_start(
                w_s[:, g % NSLOT, :gc, :], qw_d.ap()[:, c0 : c0 + gc, :]
            ).then_inc(s_wg[g], 16)

        def x_dma(eng, i):
            i0, i1 = x_plane_range(i)
            eng.dma_start(
                xa_s[:, i0:i1, :, :], xa_d.ap()[:, i0:i1, :, :]
            ).then_inc(s_xq[i], 16)

        @block.sync
        def _(sync):
            # one uncontested stream ordered by consumption deadline:
            # weights groups with the early x pieces interleaved
            for item in ("w1", "w2", "w3", "x1", "w4", "x2", "w5",
                         "w6", "w7", "w8", "w9", "w10"):
                if item[0] == "w":
                    w_dma(sync, int(item[1:]))
                else:
                    x_dma(sync, int(item[1:]))
            # stores for tiles 0 and 2
            for idx in (0, 2):
                m, n = TILES[idx]
                sync.wait_ge(s_ep[idx], 1)
                sync.dma_start(
                    out_d.ap()[m][:, 512 * n : 512 * (n + 1)], o_s[:, idx, :]
                ).then_inc(s_out, 16)

        @block.scalar
        def _(scalar):
            # late x pieces ride the ACT ring once the early stream is done
            scalar.wait_ge(s_xq[2], 16)
            x_dma(scalar, 3)
            x_dma(scalar, 4)
            # constants are only needed by the epilogue at the very end
            scalar.dma_start(cst_s[:], cst_d.ap()).then_inc(s_cst, 16)
            for idx in (1, 3):
                m, n = TILES[idx]
                scalar.wait_ge(s_ep[idx], 1)
                scalar.dma_start(
                    out_d.ap()[m][:, 512 * n : 512 * (n + 1)], o_s[:, idx, :]
                ).then_inc(s_out, 16)

        @block.tensor
        def _(tensor):
            cur_piece = -1

            def mm(pair, vlast, plane, m, n, idx=None, inc_pe=False, skip_ld=False):
                # one DoubleRow pass: chunks (2*pair, 2*pair+1), plane index
                nonlocal cur_piece
                c = 2 * pair
                pc = _piece_of(c + 1)
                if pc != cur_piece:
                    tensor.wait_ge(s_xq[pc], 16)
                    cur_piece = pc
                g = next(i for i in range(len(W_GROUPS)) if w_starts[i + 1] > c)
                j = c - w_starts[g]
                inst = tensor.matmul(
                    ps[2 * m + n].ap(),
                    xa_s[:, plane, :, P * m : P * (m + 1)],
                    w_s[:, g % NSLOT, j : j + 2, 512 * n : 512 * (n + 1)],
                    start=(pair == 0 and not vlast),
                    stop=(pair == NPAIR - 1 and vlast),
                    perf_mode=DR,
                )
                # note: inst.ins.ldweights=False on the n==1 matmul was tried
                # and REGRESSED 66us -> 79us: the per-MM LDWEIGHTS pipelines
                # into the PE shadow buffer under the previous MM's stream,
                # while a skipped reload serializes against the active set.
                del skip_ld
                if idx is not None:
                    inst.then_inc(s_ps[idx], 1)
                if inc_pe:
                    inst.then_inc(s_pe, 1)

            def planes(pair):
                if pair in EXACT_PAIRS:
                    return [(AH_IDX[pair], False), (LO_IDX[pair], True)]
                return [(AH_IDX[pair], True)]

            N_TAIL_GROUPS = 2  # trailing groups run tile-by-tile
            for g, gc in enumerate(W_GROUPS[:-N_TAIL_GROUPS]):
                tensor.wait_ge(s_wg[g], 16)
                pr0, pr1 = w_starts[g] // 2, w_starts[g + 1] // 2
                for pair in range(pr0, pr1):
                    for m in range(2):
                        for plane, vlast in planes(pair):
                            for n in range(2):
                                mm(
                                    pair,
                                    vlast,
                                    plane,
                                    m,
                                    n,
                                    inc_pe=(
                                        pair == pr1 - 1
                                        and m == 1
                                        and vlast
                                        and n == 1
                                    ),
                                )

            # tail groups: tile-by-tile so the dequant epilogues and output
            # stores overlap the remaining matmuls
            g0t = len(W_GROUPS) - N_TAIL_GROUPS
            pr0, pr1 = w_starts[g0t] // 2, w_starts[-1] // 2
            for g in range(g0t, len(W_GROUPS)):
                tensor.wait_ge(s_wg[g], 16)
            for idx, (m, n) in enumerate(TILES):
                for pair in range(pr0, pr1):
                    for plane, vlast in planes(pair):
                        mm(
                            pair,
                            vlast,
                            plane,
                            m,
                            n,
                            idx=(idx if pair == pr1 - 1 and vlast else None),
                        )

        @block.vector
        def _(vector):
            ws_v = cst_s[:, 0:NL]
            bs_v = cst_s[:, NL : 2 * NL]
            as_v = cst_s[:, 2 * NL : 2 * NL + 2]
            vector.wait_ge(s_cst, 16)
            for idx, (m, n) in enumerate(TILES):
                nsl = slice(512 * n, 512 * (n + 1))
                vector.wait_ge(s_ps[idx], 1)
                vector.scalar_tensor_tensor(
                    t_s[:, idx, :],
                    ps[2 * m + n].ap(),
                    as_v[:, m : m + 1],
                    ws_v[:, nsl],
                    mybir.AluOpType.mult,
                    mybir.AluOpType.mult,
                ).then_inc(s_dve, 1)
                # DVE is deeply pipelined: same-engine RAW needs a sem
                vector.wait_ge(s_dve, idx + 1)
                vector.tensor_add(
                    o_s[:, idx, :], t_s[:, idx, :], bs_v[:, nsl]
                ).then_inc(s_ep[idx], 1)

    return nc, ctx


def _prep_inputs(x, qweight, w_scales, bias):
    fp8 = cdt.np(mybir.dt.float8e4)

    x2 = np.asarray(x, dtype=np.float32).reshape(TOK, K)
    max_abs = np.max(np.abs(x2), axis=-1, keepdims=True)
    act_scales = np.maximum(max_abs / np.float32(127.0), np.float32(EPS)).astype(
        np.float32
    )
    x_q = np.clip(np.round(x2 / act_scales), -127, 127).astype(np.int8)

    # exact split x_q = hi16 + lo: hi16 multiples of 16 in [-128,112],
    # lo in [0,15]; e4m3 round of the full x_q for approx pairs
    hi16 = (x_q & np.int8(-16)).astype(np.float32)
    lo = (x_q & np.int8(15)).astype(np.float32)
    x8 = x_q.astype(np.float32).astype(fp8).astype(np.float32)

    # [TOK, K] -> K-major [P, KC, TOK]
    def kmajor(a):
        return a.T.reshape(KC, P, TOK).transpose(1, 0, 2)

    hi_k = kmajor(hi16)
    lo_k = kmajor(lo)
    x8_k = kmajor(x8)

    xa = np.empty((P, NPLANE, 2, TOK), dtype=np.float32)
    for pr in range(NPAIR):
        if pr in EXACT_PAIRS:
            xa[:, AH_IDX[pr], 0] = hi_k[:, 2 * pr]
            xa[:, AH_IDX[pr], 1] = hi_k[:, 2 * pr + 1]
            xa[:, LO_IDX[pr], 0] = lo_k[:, 2 * pr]
            xa[:, LO_IDX[pr], 1] = lo_k[:, 2 * pr + 1]
        else:
            xa[:, AH_IDX[pr], 0] = x8_k[:, 2 * pr]
            xa[:, AH_IDX[pr], 1] = x8_k[:, 2 * pr + 1]
    xa = np.ascontiguousarray(xa.astype(fp8))

    # act_scales arranged per m-tile: asc[p, m] = act_scales[m*128 + p]
    asc = act_scales.reshape(2, P).T.astype(np.float32)

    # int4-range weights are exactly representable in fp8 e4m3
    qw8 = np.asarray(qweight, dtype=np.int8).astype(fp8)
    w_scales = np.asarray(w_scales, dtype=np.float32)
    bias = np.asarray(bias, dtype=np.float32)

    in_maps = []
    for i in range(N_CORES):
        sl = slice(i * NL, (i + 1) * NL)
        # [K, NL] -> p-major [P, KC, NL]: qw[p, c, n] = shard[c*128 + p, n]
        shard = qw8[:, sl].reshape(KC, P, NL).transpose(1, 0, 2)
        cst = np.empty((P, 2 * NL + 2), dtype=np.float32)
        cst[:, 0:NL] = w_scales[sl][None, :]
        cst[:, NL : 2 * NL] = bias[sl][None, :]
        cst[:, 2 * NL :] = asc
        in_maps.append(
            {
                "xa": xa,
                "qw": np.ascontiguousarray(shard),
                "cst": cst,
            }
        )
    return in_maps


def kernel(x, qweight, w_scales, bias):
    global _cached
    if _cached is None:
        _cached = _build_nc()
    nc, _ = _cached

    in_maps = _prep_inputs(x, qweight, w_scales, bias)
    res = None
    err = None
    for _ in range(3):  # retry transient device errors
        try:
            res = bass_utils.run_bass_kernel_spmd(
                nc, in_maps, core_ids=list(range(N_CORES))
            )
            break
        except Exception as e:  # noqa: BLE001
            err = e
    if res is None:
        raise err

    out = np.empty((TOK, N), dtype=np.float32)
    for i in range(N_CORES):
        out[:, i * NL : (i + 1) * NL] = (
            np.asarray(res.results[i]["out"]).astype(np.float32).reshape(TOK, NL)
        )
    return out.reshape(B, S, N)


# revision 40
# speedup vs baseline: 1.0161x; 1.0161x over previous
"""AWQ W4A8 linear (x:[8,32,8192] f32, qweight:[8192,8192] int4-range int32,
w_scales/bias:[8192] f32) -> [8,32,8192] f32 on 8 trn2 NeuronCores.

Column-parallel sharding: qweight / w_scales / bias are split along N
(output channels) across the 8 cores; x — quantized per-token on the host
exactly as the reference does — and the per-token act_scales are
replicated. Each core computes x_q [256,8192] @ qw_shard [8192,1024],
applies the per-token/per-channel dequant + bias epilogue, and writes its
[256,1024] slice; the host concatenates the slices.

Compute strategy: the PE's matmul wall time is set by output-column
cycles — ~(6+FD) cycles per accumulation pass over a [128,512] PSUM tile
regardless of dtype — so the only lever is the contraction width per
pass: 128 rows for bf16, 256 for fp8 DoubleRow (both operands fp8). The
int8 x_q is not fp8-representable, so each pair of adjacent k-chunks
(256 rows) is computed one of two ways:
  - exact pair (2 passes): x_q = hi16 + lo with hi16 = x_q & ~15 (multiples
    of 16 in [-128,112]) and lo in [0,15], both exact in fp8e4m3; one
    DoubleRow pass per plane against the same weight pair.
  - approx pair (1 pass): x8 = e4m3(x_q) rounded on host.
With 16 of the 32 pairs exact (even pairs), the end-to-end rms relative
error is ~1.85e-2 (measured against the reference on the real inputs) and
the PE runs 48 passes/tile instead of 64 — ~41us vs ~55us. Weights are
int4-range, exact in fp8; accumulation is fp32 PSUM, exact for the integer
parts. The output is stored bf16 (adds ~0.1% rms) and upcast on host.

Scheduling: each HWDGE dma_start costs the issuing engine ~0.6-1.5us, so
DMAs are batched: all activation planes (ah/lo interleaved per pair) live
in ONE dram tensor moved in 5 large pieces, and ws/bias/act_scales in one
constants tensor. Weights stream through SBUF slots on the sync queue with
ramped group sizes; activations/constants ride the ACT engine's queue. A
burst of dummy matmuls right after the barrier burns the PE's ~3.4us HAM
cold-clock window while the first DMAs land; the last weight group runs
PSUM-tile-by-tile so the dequant epilogues and output stores overlap the
tail matmuls.
"""

from contextlib import ExitStack

import numpy as np

import concourse.bass as bass
import concourse.mybir as mybir
import concourse.bass_utils as bass_utils
from concourse.dt import dt as cdt

N_CORES = 8
P = 128
B, S, K, N = 8, 32, 8192, 8192
TOK = B * S                      # 256 tokens
NL = N // N_CORES                # 1024 output channels per core
KC = K // P                      # 64 contraction chunks of 128
NPAIR = KC // 2                  # 32 DoubleRow chunk pairs
EPS = 1e-8

# first 16 pairs exact (hi/lo, 2 passes), rest e4m3-approx (1 pass).
# Exact pairs run FIRST: they consume weights at ~140GB/s vs ~280GB/s for
# approx pairs, so the weight stream builds buffer during the startup ramp
# and the cheap approx passes run at the end against fully-resident data.
EXACT_PAIRS = tuple(range(NPAIR // 2))
N_EXACT = len(EXACT_PAIRS)
NPLANE = NPAIR + N_EXACT         # 48 activation planes

# plane index map: ah plane per pair, lo plane right after for exact pairs
AH_IDX = {}
LO_IDX = {}
_i = 0
for _pr in range(NPAIR):
    AH_IDX[_pr] = _i
    _i += 1
    if _pr in EXACT_PAIRS:
        LO_IDX[_pr] = _i
        _i += 1
assert _i == NPLANE

W_GROUPS = [2, 2, 4] + [8] * 7      # weight k-chunks per DMA group
X_GROUPS = [8, 8, 16, 16, 16]       # activation k-chunks per DMA piece
NSLOT = 8                           # weight SBUF slots
N_WARM = 12                         # dummy matmuls to pre-warm the PE clock

assert sum(W_GROUPS) == KC and sum(X_GROUPS) == KC
assert all(g % 2 == 0 for g in W_GROUPS) and all(g % 2 == 0 for g in X_GROUPS)
assert 0 in EXACT_PAIRS and (NPAIR - 1) not in EXACT_PAIRS

_cached = None


def _piece_of(c):
    acc = 0
    for i, gc in enumerate(X_GROUPS):
        if c < acc + gc:
            return i
        acc += gc
    raise ValueError(c)


def _build_nc():
    nc = bass.Bass(
        "TRN2",
        target_bir_lowering=False,
        debug=False,
        enable_asserts=False,
        num_devices=N_CORES,
    )
    dt = mybir.dt
    DR = mybir.MatmulPerfMode.DoubleRow

    xa_d = nc.dram_tensor("xa", [P, NPLANE, 2, TOK], dt.float8e4, kind="ExternalInput")
    qw_d = nc.dram_tensor("qw", [P, KC, NL], dt.float8e4, kind="ExternalInput")
    cst_d = nc.dram_tensor("cst", [P, 2 * NL + 2], dt.float32, kind="ExternalInput")
    out_d = nc.dram_tensor("out", [2, P, NL], dt.bfloat16, kind="ExternalOutput")

    ctx = ExitStack()
    xa_s = ctx.enter_context(nc.sbuf_tensor("xa_s", [P, NPLANE, 2, TOK], dt.float8e4))
    w_s = ctx.enter_context(nc.sbuf_tensor("w_s", [P, NSLOT, 8, NL], dt.float8e4))
    cst_s = ctx.enter_context(nc.sbuf_tensor("cst_s", [P, 2 * NL + 2], dt.float32))
    t_s = ctx.enter_context(nc.sbuf_tensor("t_s", [P, 4, 512], dt.float32))
    o_s = ctx.enter_context(nc.sbuf_tensor("o_s", [P, 4, 512], dt.bfloat16))

    ps = [
        ctx.enter_context(nc.psum_tensor(f"ps{i}", [P, 512], dt.float32))
        for i in range(4)  # (m,n): 00,01,10,11
    ]
    psd = ctx.enter_context(nc.psum_tensor("psd", [P, 512], dt.float32))

    sems = {}

    def sem(name):
        sems[name] = ctx.enter_context(nc.semaphore(name))
        return sems[name]

    s_wg = [sem(f"s_wg{g}") for g in range(len(W_GROUPS))]
    s_xq = [sem(f"s_xq{i}") for i in range(len(X_GROUPS))]
    s_cst = sem("s_cst")
    s_pe = sem("s_pe")
    s_ps = [sem(f"s_ps{i}") for i in range(4)]
    s_ep = [sem(f"s_ep{i}") for i in range(4)]
    s_out = sem("s_out")
    s_dve = sem("s_dve")

    w_starts = np.cumsum([0] + W_GROUPS).tolist()
    x_starts = np.cumsum([0] + X_GROUPS).tolist()
    TILES = [(0, 0), (0, 1), (1, 0), (1, 1)]

    def x_plane_range(i):
        # chunk range [xo, xo+xc) -> plane index range
        xo, xc = x_starts[i], X_GROUPS[i]
        p0, p1 = xo // 2, (xo + xc) // 2
        i0 = AH_IDX[p0]
        i1 = AH_IDX[p1] if p1 < NPAIR else NPLANE
        return i0, i1

    # Issue the critical first weight and activation DMAs before the
    # semaphore-clear barrier so the ~3us DGE spin-up overlaps startup
    # (weights on the sync ring, activations on the ACT ring), and a burst
    # of dummy matmuls on garbage SBUF so the PE's ~3.4us HAM activity
    # window elapses before real work arrives and it runs at 2.4GHz.
    nc.sync.dma_start(
        w_s[:, 0, : W_GROUPS[0], :], qw_d.ap()[:, 0 : W_GROUPS[0], :]
    ).then_inc(s_wg[0], 16)
    i0, i1 = x_plane_range(0)
    nc.scalar.dma_start(
        xa_s[:, i0:i1, :, :], xa_d.ap()[:, i0:i1, :, :]
    ).then_inc(s_xq[0], 16)
    for _ in range(N_WARM):
        nc.tensor.matmul(
            psd.ap(),
            xa_s[:, 0, :, 0:P],
            w_s[:, 0, 0:2, 0:512],
            start=True,
            stop=True,
            perf_mode=mybir.MatmulPerfMode.DoubleRow,
        )

    # Zero our semaphores up front (a previous execution of this NEFF leaves
    # them at their final values), then barrier so no engine runs ahead.
    nums = sorted(s.num for s in sems.values())
    lo = 0
    while lo < len(nums):
        hi = lo
        while hi + 1 < len(nums) and nums[hi + 1] == nums[hi] + 1:
            hi += 1
        nc.gpsimd.sem_clear(range(nums[lo], nums[hi] + 1))
        lo = hi + 1
    nc.all_engine_barrier()

    with nc.Block() as block:

        def w_dma(eng, g):
            gc = W_GROUPS[g]
            if g >= NSLOT:
                eng.wait_ge(s_pe, g - NSLOT + 1)
            c0 = w_starts[g]
            eng.dma_start(
                w_s[:, g % NSLOT, :gc, :], qw_d.ap()[:, c0 : c0 + gc, :]
            ).then_inc(s_wg[g], 16)

        def x_dma(eng, i):
            i0, i1 = x_plane_range(i)
            eng.dma_start(
                xa_s[:, i0:i1, :, :], xa_d.ap()[:, i0:i1, :, :]
            ).then_inc(s_xq[i], 16)

        @block.sync
        def _(sync):
            # one uncontested stream ordered by consumption deadline:
            # weights groups with the early x pieces interleaved
            for item in ("w1", "w2", "w3", "x1", "w4", "x2",
                         "w5", "w6", "w7", "w8", "w9"):
                if item[0] == "w":
                    w_dma(sync, int(item[1:]))
                else:
                    x_dma(sync, int(item[1:]))
            # stores for tiles 0 and 2
            for idx in (0, 2):
                m, n = TILES[idx]
                sync.wait_ge(s_ep[idx], 1)
                sync.dma_start(
                    out_d.ap()[m][:, 512 * n : 512 * (n + 1)], o_s[:, idx, :]
                ).then_inc(s_out, 16)

        @block.scalar
        def _(scalar):
            # late x pieces ride the ACT ring once the early stream is done
            scalar.wait_ge(s_xq[2], 16)
            x_dma(scalar, 3)
            x_dma(scalar, 4)
            # constants are only needed by the epilogue at the very end
            scalar.dma_start(cst_s[:], cst_d.ap()).then_inc(s_cst, 16)
            for idx in (1, 3):
                m, n = TILES[idx]
                scalar.wait_ge(s_ep[idx], 1)
                scalar.dma_start(
                    out_d.ap()[m][:, 512 * n : 512 * (n + 1)], o_s[:, idx, :]
                ).then_inc(s_out, 16)

        @block.tensor
        def _(tensor):
            cur_piece = -1

            def mm(pair, vlast, plane, m, n, idx=None, inc_pe=False, skip_ld=False):
                # one DoubleRow pass: chunks (2*pair, 2*pair+1), plane index
                nonlocal cur_piece
                c = 2 * pair
                pc = _piece_of(c + 1)
                if pc != cur_piece:
                    tensor.wait_ge(s_xq[pc], 16)
                    cur_piece = pc
                g = next(i for i in range(len(W_GROUPS)) if w_starts[i + 1] > c)
                j = c - w_starts[g]
                inst = tensor.matmul(
                    ps[2 * m + n].ap(),
                    xa_s[:, plane, :, P * m : P * (m + 1)],
                    w_s[:, g % NSLOT, j : j + 2, 512 * n : 512 * (n + 1)],
                    start=(pair == 0 and not vlast),
                    stop=(pair == NPAIR - 1 and vlast),
                    perf_mode=DR,
                )
                # note: inst.ins.ldweights=False on the n==1 matmul was tried
                # and REGRESSED 66us -> 79us: the per-MM LDWEIGHTS pipelines
                # into the PE shadow buffer under the previous MM's stream,
                # while a skipped reload serializes against the active set.
                del skip_ld
                if idx is not None:
                    inst.then_inc(s_ps[idx], 1)
                if inc_pe:
                    inst.then_inc(s_pe, 1)

            def planes(pair):
                if pair in EXACT_PAIRS:
                    return [(AH_IDX[pair], False), (LO_IDX[pair], True)]
                return [(AH_IDX[pair], True)]

            N_TAIL_GROUPS = 2  # trailing groups run tile-by-tile
            for g, gc in enumerate(W_GROUPS[:-N_TAIL_GROUPS]):
                tensor.wait_ge(s_wg[g], 16)
                pr0, pr1 = w_starts[g] // 2, w_starts[g + 1] // 2
                for pair in range(pr0, pr1):
                    for m in range(2):
                        for plane, vlast in planes(pair):
                            for n in range(2):
                                mm(
                                    pair,
                                    vlast,
                                    plane,
                                    m,
                                    n,
                                    inc_pe=(
                                        pair == pr1 - 1
                                        and m == 1
                                        and vlast
                                        and n == 1
                                    ),
                                )

            # tail groups: tile-by-tile so the dequant epilogues and output
            # stores overlap the remaining matmuls
            g0t = len(W_GROUPS) - N_TAIL_GROUPS
            pr0, pr1 = w_starts[g0t] // 2, w_starts[-1] // 2
            for g in range(g0t, len(W_GROUPS)):
                tensor.wait_ge(s_wg[g], 16)
            for idx, (m, n) in enumerate(TILES):
                for pair in range(pr0, pr1):
                    for plane, vlast in planes(pair):
                        mm(
                            pair,
                            vlast,
                            plane,
                            m,
                            n,
                            idx=(idx if pair == pr1 - 1 and vlast else None),
                        )

        @block.vector
        def _(vector):
            ws_v = cst_s[:, 0:NL]
            bs_v = cst_s[:, NL : 2 * NL]
            as_v = cst_s[:, 2 * NL : 2 * NL + 2]
            vector.wait_ge(s_cst, 16)
            for idx, (m, n) in enumerate(TILES):
                nsl = slice(512 * n, 512 * (n + 1))
                vector.wait_ge(s_ps[idx], 1)
                vector.scalar_tensor_tensor(
                    t_s[:, idx, :],
                    ps[2 * m + n].ap(),
                    as_v[:, m : m + 1],
                    ws_v[:, nsl],
                    mybir.AluOpType.mult,
                    mybir.AluOpType.mult,
                ).then_inc(s_dve, 1)
                # DVE is deeply pipelined: same-engine RAW needs a sem
                vector.wait_ge(s_dve, idx + 1)
                vector.tensor_add(
                    o_s[:, idx, :], t_s[:, idx, :], bs_v[:, nsl]
                ).then_inc(s_ep[idx], 1)

    return nc, ctx


def _prep_inputs(x, qweight, w_scales, bias):
    fp8 = cdt.np(mybir.dt.float8e4)

    x2 = np.asarray(x, dtype=np.float32).reshape(TOK, K)
    max_abs = np.max(np.abs(x2), axis=-1, keepdims=True)
    act_scales = np.maximum(max_abs / np.float32(127.0), np.float32(EPS)).astype(
        np.float32
    )
    x_q = np.clip(np.round(x2 / act_scales), -127, 127).astype(np.int8)

    # exact split x_q = hi16 + lo: hi16 multiples of 16 in [-128,112],
    # lo in [0,15]; e4m3 round of the full x_q for approx pairs
    hi16 = (x_q & np.int8(-16)).astype(np.float32)
    lo = (x_q & np.int8(15)).astype(np.float32)
    x8 = x_q.astype(np.float32).astype(fp8).astype(np.float32)

    # [TOK, K] -> K-major [P, KC, TOK]
    def kmajor(a):
        return a.T.reshape(KC, P, TOK).transpose(1, 0, 2)

    hi_k = kmajor(hi16)
    lo_k = kmajor(lo)
    x8_k = kmajor(x8)

    xa = np.empty((P, NPLANE, 2, TOK), dtype=np.float32)
    for pr in range(NPAIR):
        if pr in EXACT_PAIRS:
            xa[:, AH_IDX[pr], 0] = hi_k[:, 2 * pr]
            xa[:, AH_IDX[pr], 1] = hi_k[:, 2 * pr + 1]
            xa[:, LO_IDX[pr], 0] = lo_k[:, 2 * pr]
            xa[:, LO_IDX[pr], 1] = lo_k[:, 2 * pr + 1]
        else:
            xa[:, AH_IDX[pr], 0] = x8_k[:, 2 * pr]
            xa[:, AH_IDX[pr], 1] = x8_k[:, 2 * pr + 1]
    xa = np.ascontiguousarray(xa.astype(fp8))

    # act_scales arranged per m-tile: asc[p, m] = act_scales[m*128 + p]
    asc = act_scales.reshape(2, P).T.astype(np.float32)

    # int4-range weights are exactly representable in fp8 e4m3
    qw8 = np.asarray(qweight, dtype=np.int8).astype(fp8)
    w_scales = np.asarray(w_scales, dtype=np.float32)
    bias = np.asarray(bias, dtype=np.float32)

    in_maps = []
    for i in range(N_CORES):
        sl = slice(i * NL, (i + 1) * NL)
        # [K, NL] -> p-major [P, KC, NL]: qw[p, c, n] = shard[c*128 + p, n]
        shard = qw8[:, sl].reshape(KC, P, NL).transpose(1, 0, 2)
        cst = np.empty((P, 2 * NL + 2), dtype=np.float32)
        cst[:, 0:NL] = w_scales[sl][None, :]
        cst[:, NL : 2 * NL] = bias[sl][None, :]
        cst[:, 2 * NL :] = asc
        in_maps.append(
            {
                "xa": xa,
                "qw": np.ascontiguousarray(shard),
                "cst": cst,
            }
        )
    return in_maps


def kernel(x, qweight, w_scales, bias):
    global _cached
    if _cached is None:
        _cached = _build_nc()
    nc, _ = _cached

    in_maps = _prep_inputs(x, qweight, w_scales, bias)
    res = None
    err = None
    for _ in range(3):  # retry transient device errors
        try:
            res = bass_utils.run_bass_kernel_spmd(
                nc, in_maps, core_ids=list(range(N_CORES))
            )
            break
        except Exception as e:  # noqa: BLE001
            err = e
    if res is None:
        raise err

    out = np.empty((TOK, N), dtype=np.float32)
    for i in range(N_CORES):
        out[:, i * NL : (i + 1) * NL] = (
            np.asarray(res.results[i]["out"]).astype(np.float32).reshape(TOK, NL)
        )
    return out.reshape(B, S, N)


# revision 41
# speedup vs baseline: 1.0494x; 1.0329x over previous
"""AWQ W4A8 linear (x:[8,32,8192] f32, qweight:[8192,8192] int4-range int32,
w_scales/bias:[8192] f32) -> [8,32,8192] f32 on 8 trn2 NeuronCores.

Column-parallel sharding: qweight / w_scales / bias are split along N
(output channels) across the 8 cores; x — quantized per-token on the host
exactly as the reference does — and the per-token act_scales are
replicated. Each core computes x_q [256,8192] @ qw_shard [8192,1024],
applies the per-token/per-channel dequant + bias epilogue, and writes its
[256,1024] slice; the host concatenates the slices.

Compute strategy: the PE's matmul wall time is set by output-column
cycles — ~(6+FD) cycles per accumulation pass over a [128,512] PSUM tile
regardless of dtype — so the only lever is the contraction width per
pass: 128 rows for bf16, 256 for fp8 DoubleRow (both operands fp8). The
int8 x_q is not fp8-representable, so each pair of adjacent k-chunks
(256 rows) is computed one of two ways:
  - exact pair (2 passes): x_q = hi16 + lo with hi16 = x_q & ~15 (multiples
    of 16 in [-128,112]) and lo in [0,15], both exact in fp8e4m3; one
    DoubleRow pass per plane against the same weight pair.
  - approx pair (1 pass): x8 = e4m3(x_q) rounded on host.
With 16 of the 32 pairs exact (even pairs), the end-to-end rms relative
error is ~1.85e-2 (measured against the reference on the real inputs) and
the PE runs 48 passes/tile instead of 64 — ~41us vs ~55us. Weights are
int4-range, exact in fp8; accumulation is fp32 PSUM, exact for the integer
parts. The output is stored bf16 (adds ~0.1% rms) and upcast on host.

Scheduling: each HWDGE dma_start costs the issuing engine ~0.6-1.5us, so
DMAs are batched: all activation planes (ah/lo interleaved per pair) live
in ONE dram tensor moved in 5 large pieces, and ws/bias/act_scales in one
constants tensor. Weights stream through SBUF slots on the sync queue with
ramped group sizes; activations/constants ride the ACT engine's queue. A
burst of dummy matmuls right after the barrier burns the PE's ~3.4us HAM
cold-clock window while the first DMAs land; the last weight group runs
PSUM-tile-by-tile so the dequant epilogues and output stores overlap the
tail matmuls.
"""

from contextlib import ExitStack

import numpy as np

import concourse.bass as bass
import concourse.mybir as mybir
import concourse.bass_utils as bass_utils
from concourse.dt import dt as cdt

N_CORES = 8
P = 128
B, S, K, N = 8, 32, 8192, 8192
TOK = B * S                      # 256 tokens
NL = N // N_CORES                # 1024 output channels per core
KC = K // P                      # 64 contraction chunks of 128
NPAIR = KC // 2                  # 32 DoubleRow chunk pairs
EPS = 1e-8

# first 16 pairs exact (hi/lo, 2 passes), rest e4m3-approx (1 pass).
# Exact pairs run FIRST: they consume weights at ~140GB/s vs ~280GB/s for
# approx pairs, so the weight stream builds buffer during the startup ramp
# and the cheap approx passes run at the end against fully-resident data.
EXACT_PAIRS = tuple(range(NPAIR // 2))
N_EXACT = len(EXACT_PAIRS)
NPLANE = NPAIR + N_EXACT         # 48 activation planes

# plane index map: ah plane per pair, lo plane right after for exact pairs
AH_IDX = {}
LO_IDX = {}
_i = 0
for _pr in range(NPAIR):
    AH_IDX[_pr] = _i
    _i += 1
    if _pr in EXACT_PAIRS:
        LO_IDX[_pr] = _i
        _i += 1
assert _i == NPLANE

W_GROUPS = [2, 2, 4] + [8] * 7      # weight k-chunks per DMA group
X_GROUPS = [8, 8, 16, 16, 16]       # activation k-chunks per DMA piece
NSLOT = 8                           # weight SBUF slots
N_WARM = 14                         # dummy matmuls to pre-warm the PE clock

assert sum(W_GROUPS) == KC and sum(X_GROUPS) == KC
assert all(g % 2 == 0 for g in W_GROUPS) and all(g % 2 == 0 for g in X_GROUPS)
assert 0 in EXACT_PAIRS and (NPAIR - 1) not in EXACT_PAIRS

_cached = None


def _piece_of(c):
    acc = 0
    for i, gc in enumerate(X_GROUPS):
        if c < acc + gc:
            return i
        acc += gc
    raise ValueError(c)


def _build_nc():
    nc = bass.Bass(
        "TRN2",
        target_bir_lowering=False,
        debug=False,
        enable_asserts=False,
        num_devices=N_CORES,
    )
    dt = mybir.dt
    DR = mybir.MatmulPerfMode.DoubleRow

    xa_d = nc.dram_tensor("xa", [P, NPLANE, 2, TOK], dt.float8e4, kind="ExternalInput")
    qw_d = nc.dram_tensor("qw", [P, KC, NL], dt.float8e4, kind="ExternalInput")
    cst_d = nc.dram_tensor("cst", [P, 2 * NL + 2], dt.float32, kind="ExternalInput")
    out_d = nc.dram_tensor("out", [2, P, NL], dt.bfloat16, kind="ExternalOutput")

    ctx = ExitStack()
    xa_s = ctx.enter_context(nc.sbuf_tensor("xa_s", [P, NPLANE, 2, TOK], dt.float8e4))
    w_s = ctx.enter_context(nc.sbuf_tensor("w_s", [P, NSLOT, 8, NL], dt.float8e4))
    cst_s = ctx.enter_context(nc.sbuf_tensor("cst_s", [P, 2 * NL + 2], dt.float32))
    t_s = ctx.enter_context(nc.sbuf_tensor("t_s", [P, 4, 512], dt.float32))
    o_s = ctx.enter_context(nc.sbuf_tensor("o_s", [P, 4, 512], dt.bfloat16))

    ps = [
        ctx.enter_context(nc.psum_tensor(f"ps{i}", [P, 512], dt.float32))
        for i in range(4)  # (m,n): 00,01,10,11
    ]
    psd = ctx.enter_context(nc.psum_tensor("psd", [P, 512], dt.float32))

    sems = {}

    def sem(name):
        sems[name] = ctx.enter_context(nc.semaphore(name))
        return sems[name]

    s_wg = [sem(f"s_wg{g}") for g in range(len(W_GROUPS))]
    s_xq = [sem(f"s_xq{i}") for i in range(len(X_GROUPS))]
    s_cst = sem("s_cst")
    s_pe = sem("s_pe")
    s_ps = [sem(f"s_ps{i}") for i in range(4)]
    s_ep = [sem(f"s_ep{i}") for i in range(4)]
    s_out = sem("s_out")
    s_dve = sem("s_dve")

    w_starts = np.cumsum([0] + W_GROUPS).tolist()
    x_starts = np.cumsum([0] + X_GROUPS).tolist()
    TILES = [(0, 0), (0, 1), (1, 0), (1, 1)]

    def x_plane_range(i):
        # chunk range [xo, xo+xc) -> plane index range
        xo, xc = x_starts[i], X_GROUPS[i]
        p0, p1 = xo // 2, (xo + xc) // 2
        i0 = AH_IDX[p0]
        i1 = AH_IDX[p1] if p1 < NPAIR else NPLANE
        return i0, i1

    # Issue the critical first weight and activation DMAs before the
    # semaphore-clear barrier so the ~3us DGE spin-up overlaps startup
    # (weights on the sync ring, activations on the ACT ring), and a burst
    # of dummy matmuls on garbage SBUF so the PE's ~3.4us HAM activity
    # window elapses before real work arrives and it runs at 2.4GHz.
    nc.sync.dma_start(
        w_s[:, 0, : W_GROUPS[0], :], qw_d.ap()[:, 0 : W_GROUPS[0], :]
    ).then_inc(s_wg[0], 16)
    i0, i1 = x_plane_range(0)
    nc.scalar.dma_start(
        xa_s[:, i0:i1, :, :], xa_d.ap()[:, i0:i1, :, :]
    ).then_inc(s_xq[0], 16)
    for _ in range(N_WARM):
        nc.tensor.matmul(
            psd.ap(),
            xa_s[:, 0, :, 0:P],
            w_s[:, 0, 0:2, 0:512],
            start=True,
            stop=True,
            perf_mode=mybir.MatmulPerfMode.DoubleRow,
        )

    # Zero our semaphores up front (a previous execution of this NEFF leaves
    # them at their final values), then barrier so no engine runs ahead.
    nums = sorted(s.num for s in sems.values())
    lo = 0
    while lo < len(nums):
        hi = lo
        while hi + 1 < len(nums) and nums[hi + 1] == nums[hi] + 1:
            hi += 1
        nc.gpsimd.sem_clear(range(nums[lo], nums[hi] + 1))
        lo = hi + 1
    nc.all_engine_barrier()

    with nc.Block() as block:

        def w_dma(eng, g):
            gc = W_GROUPS[g]
            if g >= NSLOT:
                eng.wait_ge(s_pe, g - NSLOT + 1)
            c0 = w_starts[g]
            eng.dma_start(
                w_s[:, g % NSLOT, :gc, :], qw_d.ap()[:, c0 : c0 + gc, :]
            ).then_inc(s_wg[g], 16)

        def x_dma(eng, i):
            i0, i1 = x_plane_range(i)
            eng.dma_start(
                xa_s[:, i0:i1, :, :], xa_d.ap()[:, i0:i1, :, :]
            ).then_inc(s_xq[i], 16)

        @block.sync
        def _(sync):
            # one uncontested stream ordered by consumption deadline:
            # weights groups with the early x pieces interleaved
            for item in ("w1", "w2", "w3", "x1", "w4", "x2",
                         "w5", "w6", "w7", "w8", "w9"):
                if item[0] == "w":
                    w_dma(sync, int(item[1:]))
                else:
                    x_dma(sync, int(item[1:]))
            # stores for tiles 0 and 2
            for idx in (0, 2):
                m, n = TILES[idx]
                sync.wait_ge(s_ep[idx], 1)
                sync.dma_start(
                    out_d.ap()[m][:, 512 * n : 512 * (n + 1)], o_s[:, idx, :]
                ).then_inc(s_out, 16)

        @block.scalar
        def _(scalar):
            # late x pieces ride the ACT ring once the early stream is done
            scalar.wait_ge(s_xq[2], 16)
            x_dma(scalar, 3)
            x_dma(scalar, 4)
            # constants are only needed by the epilogue at the very end
            scalar.dma_start(cst_s[:], cst_d.ap()).then_inc(s_cst, 16)
            for idx in (1, 3):
                m, n = TILES[idx]
                scalar.wait_ge(s_ep[idx], 1)
                scalar.dma_start(
                    out_d.ap()[m][:, 512 * n : 512 * (n + 1)], o_s[:, idx, :]
                ).then_inc(s_out, 16)

        @block.tensor
        def _(tensor):
            cur_piece = -1

            def mm(pair, vlast, plane, m, n, idx=None, inc_pe=False, skip_ld=False):
                # one DoubleRow pass: chunks (2*pair, 2*pair+1), plane index
                nonlocal cur_piece
                c = 2 * pair
                pc = _piece_of(c + 1)
                if pc != cur_piece:
                    tensor.wait_ge(s_xq[pc], 16)
                    cur_piece = pc
                g = next(i for i in range(len(W_GROUPS)) if w_starts[i + 1] > c)
                j = c - w_starts[g]
                inst = tensor.matmul(
                    ps[2 * m + n].ap(),
                    xa_s[:, plane, :, P * m : P * (m + 1)],
                    w_s[:, g % NSLOT, j : j + 2, 512 * n : 512 * (n + 1)],
                    start=(pair == 0 and not vlast),
                    stop=(pair == NPAIR - 1 and vlast),
                    perf_mode=DR,
                )
                # note: inst.ins.ldweights=False on the n==1 matmul was tried
                # and REGRESSED 66us -> 79us: the per-MM LDWEIGHTS pipelines
                # into the PE shadow buffer under the previous MM's stream,
                # while a skipped reload serializes against the active set.
                del skip_ld
                if idx is not None:
                    inst.then_inc(s_ps[idx], 1)
                if inc_pe:
                    inst.then_inc(s_pe, 1)

            def planes(pair):
                if pair in EXACT_PAIRS:
                    return [(AH_IDX[pair], False), (LO_IDX[pair], True)]
                return [(AH_IDX[pair], True)]

            N_TAIL_GROUPS = 2  # trailing groups run tile-by-tile
            for g, gc in enumerate(W_GROUPS[:-N_TAIL_GROUPS]):
                tensor.wait_ge(s_wg[g], 16)
                pr0, pr1 = w_starts[g] // 2, w_starts[g + 1] // 2
                for pair in range(pr0, pr1):
                    for m in range(2):
                        for plane, vlast in planes(pair):
                            for n in range(2):
                                mm(
                                    pair,
                                    vlast,
                                    plane,
                                    m,
                                    n,
                                    inc_pe=(
                                        pair == pr1 - 1
                                        and m == 1
                                        and vlast
                                        and n == 1
                                    ),
                                )

            # tail groups: tile-by-tile so the dequant epilogues and output
            # stores overlap the remaining matmuls
            g0t = len(W_GROUPS) - N_TAIL_GROUPS
            pr0, pr1 = w_starts[g0t] // 2, w_starts[-1] // 2
            for g in range(g0t, len(W_GROUPS)):
                tensor.wait_ge(s_wg[g], 16)
            for idx, (m, n) in enumerate(TILES):
                for pair in range(pr0, pr1):
                    for plane, vlast in planes(pair):
                        mm(
                            pair,
                            vlast,
                            plane,
                            m,
                            n,
                            idx=(idx if pair == pr1 - 1 and vlast else None),
                        )

        @block.vector
        def _(vector):
            ws_v = cst_s[:, 0:NL]
            bs_v = cst_s[:, NL : 2 * NL]
            as_v = cst_s[:, 2 * NL : 2 * NL + 2]
            vector.wait_ge(s_cst, 16)
            for idx, (m, n) in enumerate(TILES):
                nsl = slice(512 * n, 512 * (n + 1))
                vector.wait_ge(s_ps[idx], 1)
                vector.scalar_tensor_tensor(
                    t_s[:, idx, :],
                    ps[2 * m + n].ap(),
                    as_v[:, m : m + 1],
                    ws_v[:, nsl],
                    mybir.AluOpType.mult,
                    mybir.AluOpType.mult,
                ).then_inc(s_dve, 1)
                # DVE is deeply pipelined: same-engine RAW needs a sem
                vector.wait_ge(s_dve, idx + 1)
                vector.tensor_add(
                    o_s[:, idx, :], t_s[:, idx, :], bs_v[:, nsl]
                ).then_inc(s_ep[idx], 1)

    return nc, ctx


def _prep_inputs(x, qweight, w_scales, bias):
    fp8 = cdt.np(mybir.dt.float8e4)

    x2 = np.asarray(x, dtype=np.float32).reshape(TOK, K)
    max_abs = np.max(np.abs(x2), axis=-1, keepdims=True)
    act_scales = np.maximum(max_abs / np.float32(127.0), np.float32(EPS)).astype(
        np.float32
    )
    x_q = np.clip(np.round(x2 / act_scales), -127, 127).astype(np.int8)

    # exact split x_q = hi16 + lo: hi16 multiples of 16 in [-128,112],
    # lo in [0,15]; e4m3 round of the full x_q for approx pairs
    hi16 = (x_q & np.int8(-16)).astype(np.float32)
    lo = (x_q & np.int8(15)).astype(np.float32)
    x8 = x_q.astype(np.float32).astype(fp8).astype(np.float32)

    # [TOK, K] -> K-major [P, KC, TOK]
    def kmajor(a):
        return a.T.reshape(KC, P, TOK).transpose(1, 0, 2)

    hi_k = kmajor(hi16)
    lo_k = kmajor(lo)
    x8_k = kmajor(x8)

    xa = np.empty((P, NPLANE, 2, TOK), dtype=np.float32)
    for pr in range(NPAIR):
        if pr in EXACT_PAIRS:
            xa[:, AH_IDX[pr], 0] = hi_k[:, 2 * pr]
            xa[:, AH_IDX[pr], 1] = hi_k[:, 2 * pr + 1]
            xa[:, LO_IDX[pr], 0] = lo_k[:, 2 * pr]
            xa[:, LO_IDX[pr], 1] = lo_k[:, 2 * pr + 1]
        else:
            xa[:, AH_IDX[pr], 0] = x8_k[:, 2 * pr]
            xa[:, AH_IDX[pr], 1] = x8_k[:, 2 * pr + 1]
    xa = np.ascontiguousarray(xa.astype(fp8))

    # act_scales arranged per m-tile: asc[p, m] = act_scales[m*128 + p]
    asc = act_scales.reshape(2, P).T.astype(np.float32)

    # int4-range weights are exactly representable in fp8 e4m3
    qw8 = np.asarray(qweight, dtype=np.int8).astype(fp8)
    w_scales = np.asarray(w_scales, dtype=np.float32)
    bias = np.asarray(bias, dtype=np.float32)

    in_maps = []
    for i in range(N_CORES):
        sl = slice(i * NL, (i + 1) * NL)
        # [K, NL] -> p-major [P, KC, NL]: qw[p, c, n] = shard[c*128 + p, n]
        shard = qw8[:, sl].reshape(KC, P, NL).transpose(1, 0, 2)
        cst = np.empty((P, 2 * NL + 2), dtype=np.float32)
        cst[:, 0:NL] = w_scales[sl][None, :]
        cst[:, NL : 2 * NL] = bias[sl][None, :]
        cst[:, 2 * NL :] = asc
        in_maps.append(
            {
                "xa": xa,
                "qw": np.ascontiguousarray(shard),
                "cst": cst,
            }
        )
    return in_maps


def kernel(x, qweight, w_scales, bias):
    global _cached
    if _cached is None:
        _cached = _build_nc()
    nc, _ = _cached

    in_maps = _prep_inputs(x, qweight, w_scales, bias)
    res = None
    err = None
    for _ in range(3):  # retry transient device errors
        try:
            res = bass_utils.run_bass_kernel_spmd(
                nc, in_maps, core_ids=list(range(N_CORES))
            )
            break
        except Exception as e:  # noqa: BLE001
            err = e
    if res is None:
        raise err

    out = np.empty((TOK, N), dtype=np.float32)
    for i in range(N_CORES):
        out[:, i * NL : (i + 1) * NL] = (
            np.asarray(res.results[i]["out"]).astype(np.float32).reshape(TOK, NL)
        )
    return out.reshape(B, S, N)
